# revision 71
# baseline (speedup 1.0000x reference)
"""MergeAttentionSubBlockFull on 8 TRN2 NeuronCores (Bass/Tile).

Math (reference):
  xn   = LayerNorm(x) * gamma + beta                       [B,T,NE]
  W_f  = U @ blockdiag(W_in).T @ M_qkv ;  b_f = b_in @ M_qkv
  qkv  = xn @ W_f + b_f ; attention over H heads
  out  = (o @ U).reshape per-model @ W_out_m.T + b_out

Kernel algebra:
  * fold gamma/beta into W_f / b_f:  W_f' = diag(gamma) U P,
    b_f' = (beta@U) P + b_in@M,  with P = blockdiag(W_in).T @ M_qkv
    (P = stack_m(W_m.T @ M_m) computed without the block-diagonal)
  * 1/sqrt(hd) folded into the q-columns of W_f' and b_f'
  * unmerge + out-proj fused:  out = o @ U2 + b_out  with
    U2[:, m*E:(m+1)*E] = U_m @ W_out_m.T

Sharding (8 cores):
  * fold: column-slice j of W_f (288 = 192 q/k + 96 v cols) -> AllGather
    (q/k cols fp32, v cols bf16); U2 = U_m@W_out_m.T computed in full on
    every core (bf16, ~35us) -- cheaper than gathering it.
  * attention/GEMMs: data-parallel over batch (8 per core)
  * LN + transpose of pairs 0-2 and the U2/v fold are emitted inside the
    AllGather flight windows to keep the PE fed.

Precision: softmax logits have std ~2e5, so the score path (LN->fold->
q,k->QK^T) needs ~fp32 accuracy: bf16 flips argmaxes, and fp32r (~13
mantissa bits, HW-measured) is marginal.  We use a 3-term bf16 hi/lo
split (x = hi + lo, x@w = hi@w_hi + hi@w_lo + lo@w_hi, ~2^-18 relative
error) which runs at 3 cycles/row vs fp32's 4.  The hi/lo splits of the
fold operands (W_in, M_qkv, U.T) are precomputed on the host and
streamed as packed hi|lo bf16 arrays (same bytes as fp32, no on-device
split cost).  The value path (v, att@v, o@U2, and the v-columns of the
fold) is plain bf16.
NOTE: float32r matmuls whose weights are reused across several matmuls
(shared LDWEIGHTS) return garbage on HW -- avoid fp32r.
"""

import numpy as np

import concourse.bacc as bacc
import concourse.bass as bass
import concourse.mybir as mybir
import concourse.tile as tile
from concourse.bass_utils import run_bass_kernel_spmd

F32 = mybir.dt.float32
F32R = mybir.dt.float32r
BF16 = mybir.dt.bfloat16
AF = mybir.ActivationFunctionType
ALU = mybir.AluOpType

B, T, NE, E, NM, H = 64, 256, 768, 768, 3, 12
HD = NE // H                      # 64
NCORES = 8
BB = B // NCORES                  # 8 batches per core
TOK = BB * T                      # 2048 tokens per core
TE = NM * E                       # 2304
JS = TE // NCORES                 # 288 fold column slice
NCH = NE // 128                   # 6 n-chunks
TCH = TE // 128                   # 18 chunks of merged dims
OCH = TE // 128                   # 18 o-chunks per model's W_in rows
PT = 2 * T                        # tokens per batch-pair

MGSZ = (NE + 1) * 96              # bf16 gather: wv block + bfv row
QKC = 192                         # q/k columns of the fold slice
VC = JS - QKC                     # v columns (96)


def build_program():
    nc = bacc.Bacc("TRN2", target_bir_lowering=False, debug=False)

    # ---------------- DRAM I/O ----------------
    x_part = nc.dram_tensor("x_part", [TOK, NE], F32, kind="ExternalInput")
    m_hl_d = nc.dram_tensor("m_hl", [NM * TE, JS + QKC], BF16,
                            kind="ExternalInput")
    w_hl_d = nc.dram_tensor("w_hl", [NM, TE, 2 * E], BF16,
                            kind="ExternalInput")
    b_in_t = nc.dram_tensor("b_in_t", [NM, 128, OCH], F32, kind="ExternalInput")
    u_hl_d = nc.dram_tensor("u_hl", [TE, 2 * NE], BF16, kind="ExternalInput")
    g_t = nc.dram_tensor("g_t", [128, NCH], F32, kind="ExternalInput")
    beta_row = nc.dram_tensor("beta_row", [1, NE], F32, kind="ExternalInput")
    b_out_row = nc.dram_tensor("b_out_row", [1, TE], F32, kind="ExternalInput")
    ut_bf = nc.dram_tensor("ut_bf", [TE, NE], BF16, kind="ExternalInput")
    wout_bf = nc.dram_tensor("wout_bf", [NM, E, E], BF16,
                             kind="ExternalInput")
    qsv_in = nc.dram_tensor("qsv", [1, JS], F32, kind="ExternalInput")
    out_part = nc.dram_tensor("out_part", [TOK, TE], F32, kind="ExternalOutput")

    ident_np = np.eye(128, dtype=np.float32)
    ident_dram = nc.inline_tensor(ident_np, name="ident_f32")
    identb_dram = nc.inline_tensor(ident_np.astype(mybir.dt.np(BF16)),
                                   name="ident_bf16")
    ones_dram = nc.inline_tensor(np.ones((1, 128), np.float32), name="ones_row")
    onesc_dram = nc.inline_tensor(np.ones((128, 1), np.float32), name="ones_col")

    with tile.TileContext(nc) as tc:
        with tc.tile_pool(name="persist", bufs=1) as pp, \
             tc.tile_pool(name="xt_p", bufs=3) as xtp, \
             tc.tile_pool(name="stat_p", bufs=6) as stp, \
             tc.tile_pool(name="z_p", bufs=4) as zp, \
             tc.tile_pool(name="xnt_p", bufs=3) as xnp:
            hoist = {"xtp": xtp, "stp": stp, "zp": zp, "xnp": xnp,
                     "pp": pp}
            ident = pp.tile([128, 128], F32, name="ident")
            identb = pp.tile([128, 128], BF16, name="identb")
            ones1 = pp.tile([1, 128], F32, name="ones1")
            onesc = pp.tile([128, 1], F32, name="onesc")
            nc.sync.dma_start(ident[:], ident_dram[:])
            nc.sync.dma_start(identb[:], identb_dram[:])
            nc.sync.dma_start(ones1[:], ones_dram[:])
            nc.sync.dma_start(onesc[:], onesc_dram[:])

            g_sb = pp.tile([128, NCH], F32, name="g_sb")
            nc.sync.dma_start(g_sb[:], g_t[:])
            wqk_hi = [pp.tile([128, 2 * NE], BF16, name=f"wqkh{c}")
                      for c in range(NCH)]
            wqk_lo = [pp.tile([128, 2 * NE], BF16, name=f"wqkl{c}")
                      for c in range(NCH)]
            wfv_bf = [pp.tile([128, E], BF16, name=f"wfv{c}") for c in range(NCH)]
            u2_sb = [pp.tile([128, TE], BF16, name=f"u2sb{c}") for c in range(NCH)]
            bfold = pp.tile([128, TCH], F32, name="bfold")
            ob_bc = pp.tile([128, TE], BF16, name="ob_bc")
            vb_bc = pp.tile([128, E], BF16, name="vb_bc")

            with tc.tile_pool(name="dramp", bufs=1, space="DRAM") as dp:
                wf_loc = dp.tile([NE + 1, 2 * QKC], BF16, name="wf_loc")
                wf_gat = dp.tile([NCORES * (NE + 1), 2 * QKC], BF16,
                                 name="wf_gat", addr_space="Shared")
                mg_loc = dp.tile([MGSZ], BF16, name="mg_loc")
                mg_gat = dp.tile([NCORES * MGSZ], BF16, name="mg_gat",
                                 addr_space="Shared")
                scr_q = dp.tile([12 * 128], BF16, name="scr_q")

                _emit_prep_and_fold(
                    nc, tc, ones1, onesc, g_sb, b_out_row, beta_row,
                    m_hl_d, w_hl_d, b_in_t, u_hl_d, ut_bf, wout_bf, qsv_in,
                    wf_loc, wf_gat, mg_loc, mg_gat, scr_q,
                    wqk_hi, wqk_lo, wfv_bf, u2_sb, bfold, ob_bc, vb_bc,
                    ident, identb, x_part, hoist)

            _emit_batches(nc, tc, ident, identb, x_part, out_part,
                          wqk_hi, wqk_lo, wfv_bf, u2_sb, bfold, ob_bc,
                          vb_bc, hoist)

    nc.compile()
    return nc


def _emit_ln_xnt(nc, hoist, pr, x_part, identb, psum_pool):
    """LayerNorm + bf16 hi/lo split + transpose for one batch-pair.

    Returns (xnt_hi, xnt_lo): bf16 [128, PT] tiles per 128-feature chunk,
    with xn = hi + lo to ~2^-18 relative accuracy.
    """
    xtp, stp, zp, xnp = (hoist["xtp"], hoist["stp"], hoist["zp"],
                         hoist["xnp"])
    zhs, zls = [], []
    for i in range(4):
        xt = xtp.tile([128, NE], F32, name="xt")
        nc.sync.dma_start(
            xt[:], x_part[pr * PT + i * 128:pr * PT + (i + 1) * 128, :])
        ssum = stp.tile([128, 1], F32, name="ssum")
        nc.vector.tensor_reduce(ssum[:], xt[:], mybir.AxisListType.X, ALU.add)
        nmu = stp.tile([128, 1], F32, name="nmu")
        nc.vector.tensor_scalar_mul(nmu[:], ssum[:], -1.0 / NE)
        z = zp.tile([128, NE], F32, name="z", bufs=3)
        sumsq = stp.tile([128, 1], F32, name="sumsq")
        nc.scalar.activation(z[:], xt[:], AF.Square, bias=nmu[:],
                             scale=1.0, accum_out=sumsq[:])
        var = stp.tile([128, 1], F32, name="var")
        nc.vector.tensor_scalar(var[:], sumsq[:], 1.0 / NE, 1e-5,
                                ALU.mult, ALU.add)
        std = stp.tile([128, 1], F32, name="std")
        nc.scalar.activation(std[:], var[:], AF.Sqrt)
        rstd = stp.tile([128, 1], F32, name="rstd")
        nc.vector.reciprocal(rstd[:], std[:])
        nmrs = stp.tile([128, 1], F32, name="nmrs")
        nc.vector.tensor_mul(nmrs[:], nmu[:], rstd[:])
        nc.scalar.activation(z[:], xt[:], AF.Identity,
                             bias=nmrs[:], scale=rstd[:])
        zh = zp.tile([128, NE], BF16, name="zh", bufs=3)
        nc.any.tensor_copy(zh[:], z[:])
        zl = zp.tile([128, NE], BF16, name="zl", bufs=3)
        nc.any.tensor_sub(zl[:], z[:], zh[:])
        zhs.append(zh)
        zls.append(zl)
    xnt_hi = [xnp.tile([128, PT], BF16, name=f"xnth{c}") for c in range(NCH)]
    xnt_lo = [xnp.tile([128, PT], BF16, name=f"xntl{c}") for c in range(NCH)]
    for i in range(4):
        for c in range(NCH):
            for zs, xnt in ((zhs, xnt_hi), (zls, xnt_lo)):
                t_ps = psum_pool.tile([128, 128], F32, name="t_ps",
                                      tag="tps", bufs=2)
                nc.tensor.matmul(t_ps[:], zs[i][:, c * 128:(c + 1) * 128],
                                 identb[:], start=True, stop=True)
                nc.any.tensor_copy(xnt[c][:, i * 128:(i + 1) * 128], t_ps[:])
    return xnt_hi, xnt_lo


def _emit_prep_and_fold(nc, tc, ones1, onesc, g_sb, b_out_row, beta_row,
                        m_hl_d, w_hl_d, b_in_t, u_hl_d, ut_bf, wout_bf, qsv_in,
                        wf_loc, wf_gat, mg_loc, mg_gat, scr_q,
                        wqk_hi, wqk_lo, wfv_bf, u2_sb, bfold, ob_bc, vb_bc,
                        ident, identb, x_part, hoist):
    with tc.tile_pool(name="fold_sb", bufs=1) as fp:

        # ---- phase 0: bias broadcasts + hoisted pair-0 LN/xnT ----
        with nc.named_scope("prep"), \
             tc.tile_pool(name="p1_sb", bufs=1) as p1p, \
             tc.tile_pool(name="ps1", bufs=1, space="PSUM") as ps1:
            bout_sb = p1p.tile([1, TE], F32, name="bout_sb")
            nc.sync.dma_start(bout_sb[:], b_out_row[:])
            brow_sb = p1p.tile([1, NE], F32, name="brow_sb")
            nc.sync.dma_start(brow_sb[:], beta_row[:])
            for i, w in enumerate([512, 512, 512, 512, 256]):
                bb_ps = ps1.tile([128, 512], F32, name="bb_ps", tag="bbps",
                                 bufs=2)
                nc.tensor.matmul(bb_ps[:, :w], ones1[:],
                                 bout_sb[:, i * 512:i * 512 + w],
                                 start=True, stop=True)
                nc.any.tensor_copy(ob_bc[:, i * 512:i * 512 + w], bb_ps[:, :w])
            beta_bc = fp.tile([128, NE], F32, name="beta_bc")
            for i, w in enumerate([512, 256]):
                bb_ps = ps1.tile([128, 512], F32, name="bb_ps", tag="bbps",
                                 bufs=2)
                nc.tensor.matmul(bb_ps[:, :w], ones1[:],
                                 brow_sb[:, i * 512:i * 512 + w],
                                 start=True, stop=True)
                nc.any.tensor_copy(beta_bc[:, i * 512:i * 512 + w], bb_ps[:, :w])

            bia_sb = []
            for m in range(NM):
                t = fp.tile([128, OCH], F32, name=f"bia{m}")
                nc.sync.dma_start(t[:], b_in_t[m])
                bia_sb.append(t)
            qsv_sb = fp.tile([1, JS], F32, name="qsv_sb")
            nc.sync.dma_start(qsv_sb[:], qsv_in[:])
            qsv_bc = fp.tile([128, JS], F32, name="qsv_bc")
            qv_ps = ps1.tile([128, JS], F32, name="qv_ps", tag="bbps", bufs=2)
            nc.tensor.matmul(qv_ps[:], ones1[:], qsv_sb[:],
                             start=True, stop=True)
            nc.any.tensor_copy(qsv_bc[:], qv_ps[:])

            hoist["pair0"] = _emit_ln_xnt(nc, hoist, 0, x_part, identb, ps1)

        # ---- phase 1: P = stack_m(W_m.T @ M_m) + bias accumulator ----
        # q/k columns (0:QKC): 3-term bf16 hi/lo; v columns (QKC:JS):
        # single bf16 product (value path tolerates bf16).
        p_hi = [fp.tile([128, QKC], BF16, name=f"ph_{mec}")
                for mec in range(TCH)]
        p_lo = [fp.tile([128, QKC], BF16, name=f"pl_{mec}")
                for mec in range(TCH)]
        p_v = [fp.tile([128, VC], BF16, name=f"pv_{mec}")
               for mec in range(TCH)]
        bacc_t = fp.tile([128, JS], F32, name="bacc_t")
        with nc.named_scope("fold_p"), \
             tc.tile_pool(name="w_stream", bufs=4) as wsp, \
             tc.tile_pool(name="m_stream", bufs=4) as msp, \
             tc.tile_pool(name="ps2", bufs=1, space="PSUM") as ps2:
            nc.vector.memset(bacc_t[:], 0.0)
            for m in range(NM):
                pm_ps = [ps2.tile([128, JS], F32, name=f"pm{m}_{ec}",
                                  tag="pmps", bufs=NCH + 1)
                         for ec in range(NCH)]
                for oc in range(OCH):
                    w_hl = wsp.tile([128, 2 * E], BF16, name="w_hl", bufs=6)
                    nc.sync.dma_start(w_hl[:],
                                      w_hl_d[m, oc * 128:(oc + 1) * 128, :])
                    w_h = w_hl[:, 0:E]
                    w_l = w_hl[:, E:2 * E]
                    m_hl = msp.tile([128, JS + QKC], BF16, name="m_hl",
                                    bufs=6)
                    nc.sync.dma_start(
                        m_hl[:],
                        m_hl_d[m * TE + oc * 128:m * TE + (oc + 1) * 128, :])
                    m_h = m_hl[:, 0:JS]
                    m_l = m_hl[:, JS:JS + QKC]
                    for ec in range(NCH):
                        wh_c = w_hl[:, ec * 128:(ec + 1) * 128]
                        wl_c = w_hl[:, E + ec * 128:E + (ec + 1) * 128]
                        nc.tensor.matmul(pm_ps[ec][:, 0:QKC],
                                         wh_c, m_hl[:, 0:QKC],
                                         start=(oc == 0), stop=False)
                        nc.tensor.matmul(pm_ps[ec][:, QKC:JS],
                                         wh_c, m_hl[:, QKC:JS],
                                         start=False, stop=False)
                        nc.tensor.matmul(pm_ps[ec][:, 0:QKC], wh_c,
                                         m_hl[:, JS:JS + QKC],
                                         start=False, stop=False)
                        nc.tensor.matmul(pm_ps[ec][:, 0:QKC],
                                         wl_c, m_hl[:, 0:QKC],
                                         start=False,
                                         stop=(oc == OCH - 1))
                    nc.vector.scalar_tensor_tensor(
                        bacc_t[:], m_hl[:, 0:JS], bia_sb[m][:, oc:oc + 1],
                        bacc_t[:], ALU.mult, ALU.add)
                for ec in range(NCH):
                    mec = m * NCH + ec
                    nc.any.tensor_copy(p_hi[mec][:], pm_ps[ec][:, 0:QKC])
                    nc.vector.tensor_sub(p_lo[mec][:], pm_ps[ec][:, 0:QKC],
                                         p_hi[mec][:])
                    nc.any.tensor_copy(p_v[mec][:], pm_ps[ec][:, QKC:JS])

        # ---- phase 2: W_fold_slice = diag(gamma*qsv) (U @ P) ; b_fold ----
        with nc.named_scope("fold_up"), \
             tc.tile_pool(name="ut_stream", bufs=3) as utp, \
             tc.tile_pool(name="ps3", bufs=1, space="PSUM") as ps3:
            bUT = fp.tile([128, TCH], F32, name="bUT")
            wf_ps = [ps3.tile([128, JS], F32, name=f"wf_{c}", tag="wfps",
                              bufs=NCH + 1) for c in range(NCH)]
            for mec in range(TCH):
                u_hl = utp.tile([128, 2 * NE], BF16, name="u_hl", bufs=4)
                nc.sync.dma_start(u_hl[:],
                                  u_hl_d[mec * 128:(mec + 1) * 128, :])
                tmp = utp.tile([128, NE], F32, name="bu_tmp", bufs=2)
                nc.vector.tensor_mul(tmp[:], u_hl[:, 0:NE], beta_bc[:])
                nc.vector.tensor_reduce(bUT[:, mec:mec + 1], tmp[:],
                                        mybir.AxisListType.X, ALU.add)
                for c in range(NCH):
                    uh_c = u_hl[:, c * 128:(c + 1) * 128]
                    ul_c = u_hl[:, NE + c * 128:NE + (c + 1) * 128]
                    nc.tensor.matmul(wf_ps[c][:, 0:QKC], uh_c, p_hi[mec][:],
                                     start=(mec == 0), stop=False)
                    nc.tensor.matmul(wf_ps[c][:, QKC:JS], uh_c, p_v[mec][:],
                                     start=False, stop=False)
                    nc.tensor.matmul(wf_ps[c][:, 0:QKC], uh_c, p_lo[mec][:],
                                     start=False, stop=False)
                    nc.tensor.matmul(wf_ps[c][:, 0:QKC], ul_c, p_hi[mec][:],
                                     start=False, stop=(mec == TCH - 1))
                nc.vector.scalar_tensor_tensor(
                    bacc_t[:, 0:QKC], p_hi[mec][:], bUT[:, mec:mec + 1],
                    bacc_t[:, 0:QKC], ALU.mult, ALU.add)
                nc.vector.scalar_tensor_tensor(
                    bacc_t[:, QKC:JS], p_v[mec][:], bUT[:, mec:mec + 1],
                    bacc_t[:, QKC:JS], ALU.mult, ALU.add)
            wf_sl = [fp.tile([128, JS], F32, name=f"wfsl{c}")
                     for c in range(NCH)]
            wv_sl = [fp.tile([128, 96], BF16, name=f"wvsl{c}")
                     for c in range(NCH)]
            for c in range(NCH):
                nc.vector.tensor_scalar_mul(wf_sl[c][:], wf_ps[c][:],
                                            g_sb[:, c:c + 1])
                nc.vector.tensor_mul(wf_sl[c][:], wf_sl[c][:], qsv_bc[:])
                w_hilo = fp.tile([128, 2 * QKC], BF16, name="whl",
                                 bufs=2)
                nc.any.tensor_copy(w_hilo[:, 0:QKC], wf_sl[c][:, 0:QKC])
                nc.any.tensor_sub(w_hilo[:, QKC:2 * QKC],
                                  wf_sl[c][:, 0:QKC], w_hilo[:, 0:QKC])
                nc.sync.dma_start(wf_loc[c * 128:(c + 1) * 128, :],
                                  w_hilo[:])
                nc.vector.tensor_copy(wv_sl[c][:], wf_sl[c][:, 192:JS])
                nc.sync.dma_start(
                    mg_loc[c * 128 * 96:(c + 1) * 128 * 96]
                        .rearrange("(p f) -> p f", p=128),
                    wv_sl[c][:])
            bf_ps = ps3.tile([1, JS], F32, name="bf_ps")
            nc.tensor.matmul(bf_ps[:], onesc[:], bacc_t[:],
                             start=True, stop=True)
            bf_sl = fp.tile([1, JS], F32, name="bf_sl")
            nc.any.tensor_copy(bf_sl[:], bf_ps[:])
            nc.vector.tensor_mul(bf_sl[:], bf_sl[:], qsv_sb[:])
            bfq_hl = fp.tile([1, 2 * QKC], BF16, name="bfq_hl")
            nc.any.tensor_copy(bfq_hl[:, 0:QKC], bf_sl[:, 0:QKC])
            nc.any.tensor_sub(bfq_hl[:, QKC:2 * QKC], bf_sl[:, 0:QKC],
                              bfq_hl[:, 0:QKC])
            nc.sync.dma_start(wf_loc[NE:NE + 1, :], bfq_hl[:])
            bfv_bf = fp.tile([1, 96], BF16, name="bfv_bf")
            nc.vector.tensor_copy(bfv_bf[:], bf_sl[:, 192:JS])
            nc.sync.dma_start(
                mg_loc[NE * 96:NE * 96 + 96]
                    .rearrange("(o a) -> o a", o=1),
                bfv_bf[:])

        # ---- phase 3: launch both AllGathers back-to-back, then compute
        # the full U2 = U_m @ W_out_m.T locally (bf16) during the flight.
        with nc.named_scope("gather"):
            nc.gpsimd.collective_compute(
                "AllGather", ALU.bypass,
                replica_groups=[list(range(NCORES))],
                ins=[wf_loc.opt()], outs=[wf_gat.opt()])
            nc.gpsimd.collective_compute(
                "AllGather", ALU.bypass,
                replica_groups=[list(range(NCORES))],
                ins=[mg_loc.opt()], outs=[mg_gat.opt()])

        with nc.named_scope("u2fold"), \
             tc.tile_pool(name="u2sbp", bufs=1) as u2p, \
             tc.tile_pool(name="r_stream", bufs=4) as rsp, \
             tc.tile_pool(name="psu2", bufs=1, space="PSUM") as psu:
            for m in range(NM):
                uts = []
                wos = []
                for ec in range(NCH):
                    utt = u2p.tile([128, NE], BF16, name=f"u2u{ec}")
                    nc.sync.dma_start(
                        utt[:],
                        ut_bf[(m * NCH + ec) * 128:(m * NCH + ec + 1) * 128, :])
                    uts.append(utt)
                    wot = rsp.tile([128, E], BF16, name=f"u2w{ec}", bufs=1)
                    nc.sync.dma_start(
                        wot[:], wout_bf[m, ec * 128:(ec + 1) * 128, :])
                    wos.append(wot)
                for nch in range(NCH):
                    u2o_ps = [psu.tile([128, 512], F32, name="u2ps0",
                                       tag="u2ps", bufs=4),
                              psu.tile([128, 256], F32, name="u2ps1",
                                       tag="u2ps", bufs=4)]
                    for ec in range(NCH):
                        lh = uts[ec][:, nch * 128:(nch + 1) * 128]
                        nc.tensor.matmul(u2o_ps[0][:], lh, wos[ec][:, 0:512],
                                         start=(ec == 0), stop=(ec == NCH - 1))
                        nc.tensor.matmul(u2o_ps[1][:], lh, wos[ec][:, 512:768],
                                         start=(ec == 0), stop=(ec == NCH - 1))
                    nc.any.tensor_copy(u2_sb[nch][:, m * E:m * E + 512],
                                       u2o_ps[0][:])
                    nc.any.tensor_copy(u2_sb[nch][:, m * E + 512:(m + 1) * E],
                                       u2o_ps[1][:])

        # ---- phase 4: consume W_fold/bias (wf gather only), pair-1 LN/xnT.
        # mg-gather consumption is deferred into the batch code so pair-0
        # qkv (which only needs wqk/bfold) is not gated on the bf16 gather.
        with nc.named_scope("consume"), \
             tc.tile_pool(name="vstage", bufs=1) as vsp, \
             tc.tile_pool(name="vb_ps", bufs=1, space="PSUM") as vps:
            NEr = NE + 1
            for c in range(NCH):
                nc.sync.dma_start(
                    wqk_hi[c][:].rearrange("p (r f) -> p r f", r=NCORES),
                    wf_gat[:].rearrange("(r x) f -> x r f", x=NEr)
                        [c * 128:(c + 1) * 128, :, 0:QKC])
                nc.sync.dma_start(
                    wqk_lo[c][:].rearrange("p (r f) -> p r f", r=NCORES),
                    wf_gat[:].rearrange("(r x) f -> x r f", x=NEr)
                        [c * 128:(c + 1) * 128, :, QKC:2 * QKC])
            # q/k bias (bf16 hi): rows NE of wf_gat -> scratch -> [12,128]
            nc.sync.dma_start(
                scr_q[:].rearrange("(a b) -> a b", a=NCORES),
                wf_gat[:].rearrange("(r x) f -> r x f", x=NEr)
                    [:, NE, 0:QKC])
            bf12 = vsp.tile([12, 128], BF16, name="bf12")
            nc.sync.dma_start(
                bf12[:], scr_q[:].rearrange("(c p) -> c p", c=12))
            bft_ps = vps.tile([128, 12], BF16, name="bft_ps")
            nc.tensor.transpose(bft_ps[:], bf12[:], identb[0:12, 0:12])
            nc.any.tensor_copy(bfold[:, 0:12], bft_ps[:])
            # pairs 1-3 LN+xnT fill the collective-wait window (and move
            # DVE work out of the PE-bound batch phase)
            hoist["pair1"] = _emit_ln_xnt(nc, hoist, 1, x_part, identb, vps)
            hoist["pair2"] = _emit_ln_xnt(nc, hoist, 2, x_part, identb, vps)

        bv_sb = hoist["pp"].tile([1, E], F32, name="bv_sb")

        def finish_mg(psum_pool):
            for c in range(NCH):
                nc.sync.dma_start(
                    wfv_bf[c][:].rearrange("p (r f) -> p r f", r=NCORES),
                    mg_gat[:].rearrange("(r a) -> r a", r=NCORES)
                        [:, c * 128 * 96:(c + 1) * 128 * 96]
                        .rearrange("r (p f) -> p r f", p=128))
            nc.gpsimd.dma_start(
                bv_sb[:].rearrange("o (r f) -> o r f", r=NCORES),
                mg_gat[:].rearrange("(r a) -> r a", r=NCORES)
                    [:, NE * 96:NE * 96 + 96]
                    .rearrange("(o r) f -> o r f", o=1))
            for i, w in enumerate([512, 256]):
                vb_psu = psum_pool.tile([128, 512], F32, name="vb_psu",
                                        tag="qo", bufs=2)
                nc.tensor.matmul(vb_psu[:, :w], ones1[:],
                                 bv_sb[:, i * 512:i * 512 + w],
                                 start=True, stop=True)
                nc.any.tensor_copy(vb_bc[:, i * 512:i * 512 + w],
                                   vb_psu[:, :w])

        hoist["finish_mg"] = finish_mg


def _emit_batches(nc, tc, ident, identb, x_part, out_part,
                  wqk_hi, wqk_lo, wfv_bf, u2_sb, bfold, ob_bc, vb_bc, hoist):
    HP = H // 2   # head pairs
    hoist_prs = (0, 1, 2)
    stp = hoist["stp"]
    with tc.tile_pool(name="qk_p", bufs=1) as qkp, \
         tc.tile_pool(name="att_p", bufs=3) as atp, \
         tc.tile_pool(name="ot_p", bufs=1) as otp, \
         tc.tile_pool(name="out_p", bufs=2) as outp, \
         tc.tile_pool(name="bps", bufs=1, space="PSUM") as bps:

        for pr in range(BB // 2):
            with nc.named_scope(f"pair{pr}"):
                if pr in hoist_prs:
                    xnt_hi, xnt_lo = hoist[f"pair{pr}"]
                else:
                    xnt_hi, xnt_lo = _emit_ln_xnt(nc, hoist, pr, x_part,
                                                  identb, bps)

                # ---- qkT projection, 3-term bf16 hi/lo (12 chunks) ----
                qk_hi = [qkp.tile([128, PT], BF16, name=f"qkh{j}")
                         for j in range(12)]
                qk_lo = [qkp.tile([128, PT], BF16, name=f"qkl{j}")
                         for j in range(12)]
                for j in range(12):
                    q_ps = bps.tile([128, PT], F32, name="q_ps", tag="qo",
                                    bufs=2)
                    for c in range(NCH):
                        wh_c = wqk_hi[c][:, j * 128:(j + 1) * 128]
                        wl_c = wqk_lo[c][:, j * 128:(j + 1) * 128]
                        nc.tensor.matmul(q_ps[:], wh_c, xnt_hi[c][:],
                                         start=(c == 0), stop=False)
                        nc.tensor.matmul(q_ps[:], wh_c, xnt_lo[c][:],
                                         start=False, stop=False)
                        nc.tensor.matmul(q_ps[:], wl_c, xnt_hi[c][:],
                                         start=False, stop=(c == NCH - 1))
                    nc.scalar.activation(qk_hi[j][:], q_ps[:], AF.Identity,
                                         bias=bfold[:, j:j + 1])
                    nc.vector.scalar_tensor_tensor(
                        qk_lo[j][:], q_ps[:], bfold[:, j:j + 1], qk_hi[j][:],
                        ALU.add, ALU.subtract)

                if pr == 0:
                    hoist["finish_mg"](bps)

                # ---- v in [token, feature] layout, bf16 (4 tok chunks) ----
                v_t = [qkp.tile([128, E], BF16, name=f"vt{i}")
                       for i in range(4)]
                for i in range(4):
                    v_ps = [bps.tile([128, 512], F32, name="v_ps0",
                                     tag="qo", bufs=2),
                            bps.tile([128, 256], F32, name="v_ps1",
                                     tag="tps", bufs=2)]
                    for c in range(NCH):
                        xc = xnt_hi[c][:, i * 128:(i + 1) * 128]
                        nc.tensor.matmul(v_ps[0][:], xc, wfv_bf[c][:, 0:512],
                                         start=(c == 0), stop=(c == NCH - 1))
                        nc.tensor.matmul(v_ps[1][:], xc, wfv_bf[c][:, 512:768],
                                         start=(c == 0), stop=(c == NCH - 1))
                    for seg, (s0, w) in enumerate([(0, 512), (512, 256)]):
                        nc.any.tensor_add(v_t[i][:, s0:s0 + w],
                                           v_ps[seg][:, 0:w],
                                           vb_bc[:, s0:s0 + w])

                # ---- attention + out GEMM per batch in the pair ----
                for bl in range(2):
                    b0 = bl * T
                    ot_sb = [otp.tile([128, T], BF16, name=f"ot{hp}")
                             for hp in range(HP)]
                    for hp in range(HP):
                        att_bf = {}
                        for qc in range(2):
                            s_ps = [bps.tile([128, T], F32, name=f"s_ps{hh}",
                                             tag="sps", bufs=3)
                                    for hh in range(2)]
                            for hh in range(2):
                                r0 = hh * 64
                                qsl = slice(b0 + qc * 128, b0 + (qc + 1) * 128)
                                ksl = slice(b0, b0 + T)
                                qh = qk_hi[hp][r0:r0 + 64, qsl]
                                ql = qk_lo[hp][r0:r0 + 64, qsl]
                                kh = qk_hi[6 + hp][r0:r0 + 64, ksl]
                                kl = qk_lo[6 + hp][r0:r0 + 64, ksl]
                                nc.tensor.matmul(s_ps[hh][:], qh, kh,
                                                 start=True, stop=False)
                                nc.tensor.matmul(s_ps[hh][:], qh, kl,
                                                 start=False, stop=False)
                                nc.tensor.matmul(s_ps[hh][:], ql, kh,
                                                 start=False, stop=True)
                            for hh in range(2):
                                nmax = stp.tile([128, 1], F32, name="nmax")
                                nc.vector.tensor_reduce(nmax[:], s_ps[hh][:],
                                                        mybir.AxisListType.X,
                                                        ALU.max, negate=True)
                                att = atp.tile([128, T], BF16, name="att",
                                               bufs=3)
                                sm = stp.tile([128, 1], F32, name="sm")
                                nc.scalar.activation(att[:], s_ps[hh][:],
                                                     AF.Exp, bias=nmax[:],
                                                     accum_out=sm[:])
                                rs = stp.tile([128, 1], F32, name="rs")
                                nc.vector.reciprocal(rs[:], sm[:])
                                abf = atp.tile([128, T], BF16, name="abf",
                                               bufs=4)
                                nc.any.tensor_scalar_mul(abf[:], att[:],
                                                        rs[:])
                                att_bf[(hh, qc)] = abf
                        o_ps = bps.tile([128, T], F32, name="o_ps", tag="ops",
                                        bufs=1)
                        for hh in range(2):
                            attT = [atp.tile([128, T], BF16, name=f"attT{kc}")
                                    for kc in range(2)]
                            for kc in range(2):
                                for qc in range(2):
                                    tr_ps = bps.tile([128, 128], F32,
                                                     name="tr_ps", tag="tps",
                                                     bufs=2)
                                    nc.tensor.matmul(
                                        tr_ps[:],
                                        att_bf[(hh, qc)][:, kc * 128:(kc + 1) * 128],
                                        identb[:], start=True, stop=True)
                                    nc.any.tensor_copy(
                                        attT[kc][:, qc * 128:(qc + 1) * 128],
                                        tr_ps[:])
                            r0 = hh * 64
                            h = 2 * hp + hh
                            for kc in range(2):
                                nc.tensor.matmul(
                                    o_ps[r0:r0 + 64, :],
                                    v_t[bl * 2 + kc][:, h * HD:(h + 1) * HD],
                                    attT[kc][:],
                                    start=(kc == 0), stop=(kc == 1),
                                    tile_position=(0, r0))
                        nc.any.tensor_copy(ot_sb[hp][:], o_ps[:])

                    # ---- out = oT.T @ U2 + b_out ----
                    for tc_ in range(2):
                        for noc, w in enumerate([512, 512, 512, 512, 256]):
                            oo_ps = bps.tile([128, 512], F32, name="oo_ps",
                                             tag="qo", bufs=2)
                            for c in range(NCH):
                                nc.tensor.matmul(
                                    oo_ps[:, 0:w],
                                    ot_sb[c][:, tc_ * 128:(tc_ + 1) * 128],
                                    u2_sb[c][:, noc * 512:noc * 512 + w],
                                    start=(c == 0), stop=(c == NCH - 1))
                            ou = outp.tile([128, 512], F32, name="ou")
                            nc.any.tensor_add(
                                ou[:, 0:w], oo_ps[:, 0:w],
                                ob_bc[:, noc * 512:noc * 512 + w])
                            nc.sync.dma_start(
                                out_part[(pr * 2 + bl) * T + tc_ * 128:
                                         (pr * 2 + bl) * T + (tc_ + 1) * 128,
                                         noc * 512:noc * 512 + w],
                                ou[:, 0:w])


_CACHE = {}


def _get_program():
    if "nc" not in _CACHE:
        _CACHE["nc"] = build_program()
    return _CACHE["nc"]


def build_in_maps(x, ln_gamma, ln_beta, in_proj_weight, in_proj_bias,
                  out_proj_weight, out_proj_bias, U, M_qkv, num_heads):
    x = np.asarray(x, np.float32)
    ln_gamma = np.asarray(ln_gamma, np.float32)
    ln_beta = np.asarray(ln_beta, np.float32)
    in_proj_weight = np.asarray(in_proj_weight, np.float32)
    in_proj_bias = np.asarray(in_proj_bias, np.float32)
    out_proj_weight = np.asarray(out_proj_weight, np.float32)
    out_proj_bias = np.asarray(out_proj_bias, np.float32)
    U = np.asarray(U, np.float32)
    M_qkv = np.asarray(M_qkv, np.float32)
    assert int(num_heads) == H

    u_t = np.ascontiguousarray(U.T)                       # [TE, NE]
    b_in_t = np.ascontiguousarray(
        in_proj_bias.reshape(NM, OCH, 128).transpose(0, 2, 1))
    g_t = np.ascontiguousarray(ln_gamma.reshape(NCH, 128).T)
    beta_row = np.ascontiguousarray(ln_beta.reshape(1, NE))
    b_out_row = np.ascontiguousarray(out_proj_bias.reshape(1, TE))
    w_out_t = np.ascontiguousarray(out_proj_weight.transpose(0, 2, 1))

    bf = mybir.dt.np(BF16)
    u_hi = u_t.astype(bf)
    u_lo = (u_t - u_hi.astype(np.float32)).astype(bf)
    w_hi = in_proj_weight.astype(bf)
    w_lo = (in_proj_weight - w_hi.astype(np.float32)).astype(bf)
    shared = {
        "w_hl": np.ascontiguousarray(np.concatenate([w_hi, w_lo], axis=2)),
        "b_in_t": b_in_t,
        "g_t": g_t, "beta_row": beta_row, "b_out_row": b_out_row,
        "u_hl": np.ascontiguousarray(
            np.concatenate([u_hi, u_lo], axis=1)),
        "ut_bf": np.ascontiguousarray(u_hi),
        "wout_bf": np.ascontiguousarray(w_out_t.astype(bf)),
    }
    in_maps = []
    for core in range(NCORES):
        jcols = np.concatenate([np.arange(192 * core, 192 * (core + 1)),
                                2 * NE + np.arange(96 * core, 96 * (core + 1))])
        qsv = np.where(jcols < NE, np.float32(1.0 / np.sqrt(HD)),
                       np.float32(1.0)).reshape(1, JS).astype(np.float32)
        msl = M_qkv[:, jcols]
        msl_hi = msl.astype(bf)
        msl_lo = (msl - msl_hi.astype(np.float32))[:, 0:QKC].astype(bf)
        in_maps.append({
            **shared,
            "x_part": np.ascontiguousarray(
                x[core * BB:(core + 1) * BB].reshape(TOK, NE)),
            "m_hl": np.ascontiguousarray(
                np.concatenate([msl_hi, msl_lo], axis=1)),
            "qsv": qsv,
        })
    return in_maps


def kernel(**inputs):
    nc = _get_program()
    in_maps = build_in_maps(**inputs)
    res = run_bass_kernel_spmd(nc, in_maps, list(range(NCORES)))
    out = np.empty((B, T, TE), np.float32)
    for core in range(NCORES):
        out[core * BB:(core + 1) * BB] = \
            res.results[core]["out_part"].reshape(BB, T, TE)
    return out

